# revision 30
# baseline (speedup 1.0000x reference)
"""ColBERT pairwise + in-batch negative CE loss on 8 Trainium2 NeuronCores.

Problem shapes (hardcoded): B=64, N=32, S=256, D=128, fp32.

Per core: 8 docs (c dim sharded), all 2048 query rows. 16 "units" of
[128 q-rows x 8 docs x 256 s] fp32 scores land in PSUM (bufs=2) and must
leave through the only two PSUM-capable engines:
  - DVE reduce_max direct from PSUM  (~2.26us / unit)
  - ACT copy -> f16 arena (~1.9us) + DVE f16 tensor_max tree (~1.2us)
Drain total ~48 engine-us over 2 engines => ~24us balanced floor.

Why the structure looks the way it does (all verified on this stack):
  - Dual-PSUM-operand tensor ops, gpsimd PSUM access, DMA-from-PSUM,
    uint64 ACT copies, f16 matmul PSUM output (TRN3-only), and custom-DVE
    perf modes are all rejected by the BIR verifier / ISA checks, so the
    two-engine drain above is the whole design space; its ~24us/engine
    balanced floor bounds the kernel.
  - The tile scheduler reorders instructions (readiness + priority), so
    emission order is a hint; structure is set via dependencies.
  - 7 consolidated input DMAs in priority order (first query chunk + dT
    halves first) on the fast HWDGE queues (sync/SP + scalar/ACT); only
    tiny qp rides gpsimd's slow SWDGE path.  A dummy 1-elem scalar.copy
    pulls the implicit 1.28us ACT_TABLE_LOAD into the DMA phase.
  - Warm matmul chain covers the DMA phase so the PE p-state ramp (3us
    continuous busy -> 2.4GHz) is mostly done before real matmuls.
  - Drain plan: unit0 = ACT copy split in halves (earliest possible ACT
    start, after 2 of its 4 matmuls); unit3 = halved DVE reduce (early
    vector work, placed so its PSUM-read WAR hazard doesn't stall the
    phase-in); V_MID units fill DVE gaps between tree batches; the last
    tree batch is a solo unit so only ~1.5us of f16 work trails the
    final ACT copy (larger trailing batches measured +2us).
  - No on-device n-sum: maxall [128,132] f16 is DMA'd out in two chunks
    (cols 0:64 mid-kernel, 64:132 at the end); the host does the n-sum,
    block remap, diag masking and softplus epilogue (the old on-device
    ones-matmul epilogue serialized ~0.5us behind the last reduce).
  - Measured exec on this stack is noisy (+-2us run-to-run); min ~50.2us,
    median ~52us over 12 runs of this config.

maxall layout: col block m (8 cols) = query chunk m's 8 doc maxes
(row p = q-row p of the chunk, col 8m+c = local doc c). cols 128/129 =
pairwise-neg maxes (local b = 4g+j at row 32j+n, col 128+g).
"""

import sys

import numpy as np


def _ensure_path():
    try:
        import concourse  # noqa: F401
    except ImportError:
        sys.path.insert(0, "/opt/trn_rl_repo")


_ensure_path()

import concourse.bacc as bacc  # noqa: E402
import concourse.mybir as mybir  # noqa: E402
from concourse.bass_utils import run_bass_kernel_spmd  # noqa: E402
from concourse.tile import TileContext  # noqa: E402

B, N, S, D = 64, 32, 256, 128
NC = 8
CL = B // NC  # docs / queries per core (8)
BN = B * N  # 2048 query rows
DCOLS = CL * S  # 2048 doc columns per core
NEG_INF_DIAG = 1000000.0

F32 = mybir.dt.float32
F16 = mybir.dt.float16
MMDT = mybir.dt.float16

_CACHE = {}

# ---- drain schedule ------------------------------------------------------
# unit index = query chunk m = maxall col block m.
A_SPLIT = (0, 1)    # A-units whose copies split in halves: their first
                    # halves depend only on the FIRST dT DMA, so the ACT
                    # chain starts ~1.5us earlier and never stalls on the
                    # second dT half landing
V_HEAD = 3          # halved direct reduce; late enough not to stall the
                    # PE/ACT phase-in with its PSUM-read WAR hazard
V_MID = (8, 10)     # direct DVE reduces, clustered at alternating PSUM
                    # rotation slots right after V_HEAD so their tile-WAR
                    # stalls land on DVE's own chain, not the ACT chain
TREE_BATCHES = [[0, 1, 2], [4, 5, 6], [7, 9, 11], [12, 13, 14], [15]]
A_UNITS = [m for b in TREE_BATCHES for m in b]
# maxall col block per unit, in DRAIN order: tree-batch units first (so each
# batch's 8-col blocks are contiguous and one batched reduce_max writes them
# all), then the direct-reduce units.  The host remaps via BLK.
BLK = {}
for _m in A_UNITS:
    BLK[_m] = len(BLK)
for _m in (V_HEAD,) + V_MID:
    BLK[_m] = len(BLK)
K_WARM = 5          # warm matmuls (1 low + rest mid ~ covers DMA phase)


def _install_ntff_shim():
    """Best-effort: register the axon NTFF profile hook so BASS_TRACE=1
    produces hardware profiles.  Safe no-op when unavailable."""
    try:
        import types

        import antenv

        if "antenv.axon_hooks" in sys.modules:
            return
        import trn_agent_boot.trn_boot as tb

        mod = types.ModuleType("antenv.axon_hooks")
        _hook = [None]
        mod.set_axon_ntff_profile_hook = lambda h: _hook.__setitem__(0, h)
        mod.get_axon_ntff_profile_hook = lambda: _hook[0]
        sys.modules["antenv.axon_hooks"] = mod
        antenv.axon_hooks = mod
        mod.set_axon_ntff_profile_hook(
            tb._ntff_profile_via_ctypes("/opt/axon/libaxon_pjrt.so")
        )
    except Exception:
        pass


def _build():
    nc = bacc.Bacc("TRN2", target_bir_lowering=False, debug=False, num_devices=NC)
    qT = nc.dram_tensor("qT", [D, BN], MMDT, kind="ExternalInput")
    dT = nc.dram_tensor("dT", [D, DCOLS], MMDT, kind="ExternalInput")
    nT = nc.dram_tensor("nT", [D, DCOLS], MMDT, kind="ExternalInput")
    qp = nc.dram_tensor("qp", [D, CL * N], MMDT, kind="ExternalInput")
    out_d = nc.dram_tensor("out", [128, 132], F16, kind="ExternalOutput")

    X = mybir.AxisListType.X

    with TileContext(nc) as tc:
        with (
            tc.tile_pool(name="sb", bufs=1) as sb,
            tc.tile_pool(name="ar", bufs=3) as arp,
            tc.tile_pool(name="tr", bufs=2) as trp,
            tc.tile_pool(name="ps", bufs=2, space="PSUM") as ps,
        ):
            qs = sb.tile([D, BN], MMDT, tag="qs")
            ds = sb.tile([D, DCOLS], MMDT, tag="ds")
            ns = sb.tile([D, DCOLS], MMDT, tag="ns")
            qps = sb.tile([D, CL * N], MMDT, tag="qps")
            maxall = sb.tile([128, 132], F16, tag="maxall")

            # Warm-up chain: memset-backed matmuls, no DMA dependency, keep
            # the PE continuously busy so the p-state ramp finishes before
            # the first real matmul.
            wa = sb.tile([D, 128], F16, tag="wa")
            wb = sb.tile([D, 512], F16, tag="wb")
            nc.gpsimd.memset(wa[:, :], 0.0)
            nc.gpsimd.memset(wb[:, :], 0.0)
            wt = ps.tile([128, 2048], F32, tag="chunk", name="warm")
            for w in range(K_WARM):
                nc.tensor.matmul(
                    wt[:, 512 * (w % 4) : 512 * (w % 4 + 1)],
                    wa[:, :],
                    wb[:, :],
                    start=True,
                    stop=True,
                )

            # Dummy 1-elem scalar copy: forces the implicit ACT_TABLE_LOAD
            # to be inserted here (runs during the DMA phase), so the first
            # real PSUM copy isn't delayed by the 1.28us table load.
            nc.scalar.copy(wb[0:1, 0:1], wa[0:1, 0:1])

            # Input DMAs, priority order.  The HWDGE queues (sync/SP and
            # scalar/ACT) are much faster to issue than gpsimd's SWDGE, so
            # the critical tensors (first query chunks + dT) go there; only
            # the tiny qp rides SWDGE.
            nc.sync.dma_start(out=qs[:, 0:512], in_=qT[:, 0:512])
            nc.scalar.dma_start(out=ds[:, 0:1024], in_=dT[:, 0:1024])
            nc.sync.dma_start(out=ds[:, 1024:2048], in_=dT[:, 1024:2048])
            nc.scalar.dma_start(out=qs[:, 512:1024], in_=qT[:, 512:1024])
            nc.sync.dma_start(out=ns[:, :], in_=nT[:, :])
            nc.gpsimd.dma_start(out=qps[:, :], in_=qp[:, :])
            nc.sync.dma_start(out=qs[:, 1024:2048], in_=qT[:, 1024:2048])

            arenas = {}
            bat_of = {}
            for bi, bb in enumerate(TREE_BATCHES):
                for sl, mm in enumerate(bb):
                    bat_of[mm] = (bi, sl)

            def get_arena(bi):
                if bi not in arenas:
                    arenas[bi] = arp.tile(
                        [128, 8192], F16, tag="arena", name=f"a{bi}"
                    )
                return arenas[bi]

            def emit_mms(m, t):
                bi, slot = bat_of.get(m, (None, None))
                for u in range(4):
                    nc.tensor.matmul(
                        t[:, 512 * u : 512 * (u + 1)],
                        qs[:, 128 * m : 128 * (m + 1)],
                        ds[:, 512 * u : 512 * (u + 1)],
                        start=True,
                        stop=True,
                    )
                    if m == V_HEAD and u == 1:
                        nc.vector.reduce_max(
                            maxall[:, 8 * BLK[m] : 8 * BLK[m] + 4],
                            t[:, 0:1024].rearrange("p (g s) -> p g s", s=S),
                            axis=X,
                        )
                    if m in A_SPLIT and u == 1:
                        nc.scalar.copy(
                            get_arena(bi)[:, 2048 * slot : 2048 * slot + 1024],
                            t[:, 0:1024],
                        )

            def emit_unit(m):
                t = ps.tile([128, 2048], F32, tag="chunk", name=f"u{m}")
                emit_mms(m, t)
                if m == V_HEAD:
                    nc.vector.reduce_max(
                        maxall[:, 8 * BLK[m] + 4 : 8 * BLK[m] + 8],
                        t[:, 1024:2048].rearrange("p (g s) -> p g s", s=S),
                        axis=X,
                    )
                elif m in V_MID:
                    nc.vector.reduce_max(
                        maxall[:, 8 * BLK[m] : 8 * BLK[m] + 8],
                        t[:, :].rearrange("p (g s) -> p g s", s=S),
                        axis=X,
                    )
                elif m in A_SPLIT:
                    bi, slot = bat_of[m]
                    nc.scalar.copy(
                        get_arena(bi)[:, 2048 * slot + 1024 : 2048 * (slot + 1)],
                        t[:, 1024:2048],
                    )
                else:
                    bi, slot = bat_of[m]
                    nc.scalar.copy(
                        get_arena(bi)[:, 2048 * slot : 2048 * (slot + 1)], t[:, :]
                    )

            s1outs = {}

            def get_s1out(bi):
                if bi not in s1outs:
                    s1outs[bi] = trp.tile([128, 4096], F16, tag="t1",
                                          name=f"t1_{bi}")
                return s1outs[bi]

            def emit_tree_s1(bi, sl):
                # stage 1 for one unit of batch bi (per-unit form, used for
                # the trailing batches so each can start as its copy lands)
                gv = arenas[bi][:, 2048 * sl : 2048 * (sl + 1)].rearrange(
                    "p (g s) -> p g s", s=256
                )
                ov = get_s1out(bi)[:, 1024 * sl : 1024 * (sl + 1)].rearrange(
                    "p (g s) -> p g s", s=128
                )
                nc.vector.tensor_max(ov, gv[:, :, 0:128], gv[:, :, 128:256])

            def emit_tree_rest(bi):
                L = len(TREE_BATCHES[bi])
                base = BLK[TREE_BATCHES[bi][0]]
                t1v = get_s1out(bi)[:, 0 : 1024 * L].rearrange(
                    "p (g s) -> p g s", s=128
                )
                t2 = trp.tile([128, 2048], F16, tag="t2")
                t2v = t2[:, 0 : 512 * L].rearrange("p (g s) -> p g s", s=64)
                nc.vector.tensor_max(t2v, t1v[:, :, 0:64], t1v[:, :, 64:128])
                t3 = trp.tile([128, 1024], F16, tag="t3")
                t3v = t3[:, 0 : 256 * L].rearrange("p (g s) -> p g s", s=32)
                nc.vector.tensor_max(t3v, t2v[:, :, 0:32], t2v[:, :, 32:64])
                t4 = trp.tile([128, 512], F16, tag="t4")
                t4v = t4[:, 0 : 128 * L].rearrange("p (g s) -> p g s", s=16)
                nc.vector.tensor_max(t4v, t3v[:, :, 0:16], t3v[:, :, 16:32])
                # drain-order blocks are contiguous: one batched reduce
                nc.vector.reduce_max(
                    maxall[:, 8 * base : 8 * (base + L)], t4v, axis=X
                )

            def emit_tree(bi):
                # batched stage 1: one op over the whole arena
                L = len(TREE_BATCHES[bi])
                gv = arenas[bi][:, 0 : 2048 * L].rearrange(
                    "p (g s) -> p g s", s=256
                )
                ov = get_s1out(bi)[:, 0 : 1024 * L].rearrange(
                    "p (g s) -> p g s", s=128
                )
                nc.vector.tensor_max(ov, gv[:, :, 0:128], gv[:, :, 128:256])
                emit_tree_rest(bi)

            def emit_pairwise():
                pt = ps.tile([128, 2048], F32, tag="chunk", name="ptw")
                for b in range(CL):
                    g, j = divmod(b, 4)
                    nc.tensor.matmul(
                        pt[32 * j : 32 * (j + 1), 256 * g : 256 * (g + 1)],
                        qps[:, 32 * b : 32 * (b + 1)],
                        ns[:, 256 * b : 256 * (b + 1)],
                        start=True,
                        stop=True,
                        tile_position=(0, 32 * j),
                    )
                nc.vector.reduce_max(
                    maxall[:, 128:130],
                    pt[:, 0:512].rearrange("p (g s) -> p g s", s=S),
                    axis=X,
                )

            # ---- emission order (allocation order = PSUM rotation) -----
            # pw / Vh / V8 / V10 sit at alternating rotation slots: each
            # vector PSUM read's WAR stall is absorbed by the next vector
            # read on the same tile while A-copies continue on the other.
            emit_unit(0)          # A-split: ACT starts after 2 of 4 MMs
            emit_unit(1)          # A-split
            emit_unit(2)          # A
            emit_pairwise()       # early DVE filler (needs ns+qps)
            emit_unit(3)          # Vh: halved DVE reduce
            emit_unit(4)          # A (other tile, keeps ACT fed)
            emit_unit(8)          # V  (same tile as Vh)
            emit_unit(5)          # A
            emit_unit(10)         # V  (same tile as V8)
            emit_unit(6)          # A
            emit_tree(0)          # B0 {0,1,2}
            emit_unit(7)
            emit_tree(1)          # B1 {4,5,6}
            emit_unit(9)
            emit_unit(11)
            emit_tree(2)          # B2 {7,9,11}
            nc.sync.dma_start(out=out_d[:, 0:64], in_=maxall[:, 0:64])
            emit_unit(12)
            emit_tree_s1(3, 0)
            emit_unit(13)
            emit_tree_s1(3, 1)
            emit_unit(14)
            emit_tree_s1(3, 2)
            emit_tree_rest(3)     # B3 {12,13,14} closes before last copy
            emit_unit(15)
            emit_tree(4)          # solo {15}: ~1.5us tail after last copy
            nc.sync.dma_start(out=out_d[:, 64:132], in_=maxall[:, 64:132])

    nc.finalize()
    return nc


LAST_RESULT = None


def kernel(query_embeddings, doc_embeddings, neg_doc_embeddings):
    global LAST_RESULT
    _install_ntff_shim()

    q = np.asarray(query_embeddings, dtype=np.float32)
    d = np.asarray(doc_embeddings, dtype=np.float32)
    g = np.asarray(neg_doc_embeddings, dtype=np.float32)
    assert q.shape == (B, N, D) and d.shape == (B, S, D) and g.shape == (B, S, D)

    qT_all = np.ascontiguousarray(
        q.transpose(2, 0, 1).reshape(D, BN).astype(np.float16)
    )

    in_maps = []
    for k in range(NC):
        dT_k = np.ascontiguousarray(
            d[CL * k : CL * (k + 1)].transpose(2, 0, 1).reshape(D, DCOLS)
            .astype(np.float16)
        )
        nT_k = np.ascontiguousarray(
            g[CL * k : CL * (k + 1)].transpose(2, 0, 1).reshape(D, DCOLS)
            .astype(np.float16)
        )
        qp_k = np.ascontiguousarray(qT_all[:, CL * N * k : CL * N * (k + 1)])
        in_maps.append({"qT": qT_all, "dT": dT_k, "nT": nT_k, "qp": qp_k})

    if "nc" not in _CACHE:
        _CACHE["nc"] = _build()
    res = run_bass_kernel_spmd(_CACHE["nc"], in_maps, core_ids=list(range(NC)))
    LAST_RESULT = res

    # Host epilogue: n-sum, assembly, softplus means.
    scores = np.empty((B, B), dtype=np.float64)
    negpair = np.empty((B,), dtype=np.float64)
    for k in range(NC):
        o = res.results[k]["out"].astype(np.float64)  # (128, 132)
        for m in range(16):
            # rows 32j+n, block col 8*BLK[m]+c  ->  scores[4m+j, CL*k+c]
            b8 = 8 * BLK[m]
            blk = o[:, b8 : b8 + 8].reshape(4, N, CL).sum(axis=1)
            scores[4 * m : 4 * m + 4, CL * k : CL * (k + 1)] = blk
        pw = o[:, 128:130].reshape(4, N, 2).sum(axis=1)  # rows j, col g
        for gcol in range(2):
            for j in range(4):
                negpair[CL * k + 4 * gcol + j] = pw[j, gcol]

    pos = np.diagonal(scores)
    l1 = np.logaddexp(0.0, negpair - pos).mean()
    neg_ib = (scores - np.eye(B, dtype=np.float64) * NEG_INF_DIAG).max(axis=1)
    l2 = np.logaddexp(0.0, neg_ib - pos).mean()
    return np.asarray((l1 + l2) / 2.0, dtype=np.float32)


# revision 31
# speedup vs baseline: 1.0244x; 1.0244x over previous
"""ColBERT pairwise + in-batch negative CE loss on 8 Trainium2 NeuronCores.

Problem shapes (hardcoded): B=64, N=32, S=256, D=128, fp32.

Per core: 8 docs (c dim sharded), all 2048 query rows. 16 "units" of
[128 q-rows x 8 docs x 256 s] fp32 scores land in PSUM (bufs=2) and must
leave through the only two PSUM-capable engines:
  - DVE reduce_max direct from PSUM  (~2.26us / unit)
  - ACT copy -> f16 arena (~1.9us) + DVE f16 tensor_max tree (~1.2us)
Drain total ~48 engine-us over 2 engines => ~24us balanced floor.

Why the structure looks the way it does (all verified on this stack):
  - Dual-PSUM-operand tensor ops, gpsimd PSUM access, DMA-from-PSUM,
    uint64 ACT copies, f16 matmul PSUM output (TRN3-only), and custom-DVE
    perf modes are all rejected by the BIR verifier / ISA checks, so the
    two-engine drain above is the whole design space; its ~24us/engine
    balanced floor bounds the kernel.
  - The tile scheduler reorders instructions (readiness + priority), so
    emission order is a hint; structure is set via dependencies.
  - 7 consolidated input DMAs in priority order (first query chunk + dT
    halves first) on the fast HWDGE queues (sync/SP + scalar/ACT); only
    tiny qp rides gpsimd's slow SWDGE path.  A dummy 1-elem scalar.copy
    pulls the implicit 1.28us ACT_TABLE_LOAD into the DMA phase.
  - Warm matmul chain covers the DMA phase so the PE p-state ramp (3us
    continuous busy -> 2.4GHz) is mostly done before real matmuls.
  - Drain plan: unit0 = ACT copy split in halves (earliest possible ACT
    start, after 2 of its 4 matmuls); unit3 = halved DVE reduce (early
    vector work, placed so its PSUM-read WAR hazard doesn't stall the
    phase-in); V_MID units fill DVE gaps between tree batches; the last
    tree batch is a solo unit so only ~1.5us of f16 work trails the
    final ACT copy (larger trailing batches measured +2us).
  - No on-device n-sum: maxall [128,132] f16 is DMA'd out in two chunks
    (cols 0:64 mid-kernel, 64:132 at the end); the host does the n-sum,
    block remap, diag masking and softplus epilogue (the old on-device
    ones-matmul epilogue serialized ~0.5us behind the last reduce).
  - Measured exec on this stack is noisy (+-2us run-to-run); min ~50.2us,
    median ~52us over 12 runs of this config.

maxall layout: col block m (8 cols) = query chunk m's 8 doc maxes
(row p = q-row p of the chunk, col 8m+c = local doc c). cols 128/129 =
pairwise-neg maxes (local b = 4g+j at row 32j+n, col 128+g).
"""

import sys

import numpy as np


def _ensure_path():
    try:
        import concourse  # noqa: F401
    except ImportError:
        sys.path.insert(0, "/opt/trn_rl_repo")


_ensure_path()

import concourse.bacc as bacc  # noqa: E402
import concourse.mybir as mybir  # noqa: E402
from concourse.bass_utils import run_bass_kernel_spmd  # noqa: E402
from concourse.tile import TileContext  # noqa: E402

B, N, S, D = 64, 32, 256, 128
NC = 8
CL = B // NC  # docs / queries per core (8)
BN = B * N  # 2048 query rows
DCOLS = CL * S  # 2048 doc columns per core
NEG_INF_DIAG = 1000000.0

F32 = mybir.dt.float32
F16 = mybir.dt.float16
MMDT = mybir.dt.float16

_CACHE = {}

# ---- drain schedule ------------------------------------------------------
# unit index = query chunk m = maxall col block m.
A_SPLIT = (0, 1)    # A-units whose copies split in halves: their first
                    # halves depend only on the FIRST dT DMA, so the ACT
                    # chain starts ~1.5us earlier and never stalls on the
                    # second dT half landing
V_HEAD = 3          # halved direct reduce; late enough not to stall the
                    # PE/ACT phase-in with its PSUM-read WAR hazard
V_MID = (8, 13)     # direct DVE reduces slotted into tree gaps
TREE_BATCHES = [[0, 1, 2], [4, 5, 6], [7, 9, 10], [11, 12, 14], [15]]
A_UNITS = [m for b in TREE_BATCHES for m in b]
# maxall col block per unit, in DRAIN order: tree-batch units first (so each
# batch's 8-col blocks are contiguous and one batched reduce_max writes them
# all), then the direct-reduce units.  The host remaps via BLK.
BLK = {}
for _m in A_UNITS:
    BLK[_m] = len(BLK)
for _m in (V_HEAD,) + V_MID:
    BLK[_m] = len(BLK)
K_WARM = 5          # warm matmuls (1 low + rest mid ~ covers DMA phase)


def _install_ntff_shim():
    """Best-effort: register the axon NTFF profile hook so BASS_TRACE=1
    produces hardware profiles.  Safe no-op when unavailable."""
    try:
        import types

        import antenv

        if "antenv.axon_hooks" in sys.modules:
            return
        import trn_agent_boot.trn_boot as tb

        mod = types.ModuleType("antenv.axon_hooks")
        _hook = [None]
        mod.set_axon_ntff_profile_hook = lambda h: _hook.__setitem__(0, h)
        mod.get_axon_ntff_profile_hook = lambda: _hook[0]
        sys.modules["antenv.axon_hooks"] = mod
        antenv.axon_hooks = mod
        mod.set_axon_ntff_profile_hook(
            tb._ntff_profile_via_ctypes("/opt/axon/libaxon_pjrt.so")
        )
    except Exception:
        pass


def _build():
    nc = bacc.Bacc("TRN2", target_bir_lowering=False, debug=False, num_devices=NC)
    qT = nc.dram_tensor("qT", [D, BN], MMDT, kind="ExternalInput")
    dT = nc.dram_tensor("dT", [D, DCOLS], MMDT, kind="ExternalInput")
    nT = nc.dram_tensor("nT", [D, DCOLS], MMDT, kind="ExternalInput")
    qp = nc.dram_tensor("qp", [D, CL * N], MMDT, kind="ExternalInput")
    out_d = nc.dram_tensor("out", [128, 132], F16, kind="ExternalOutput")

    X = mybir.AxisListType.X

    with TileContext(nc) as tc:
        with (
            tc.tile_pool(name="sb", bufs=1) as sb,
            tc.tile_pool(name="ar", bufs=3) as arp,
            tc.tile_pool(name="tr", bufs=2) as trp,
            tc.tile_pool(name="ps", bufs=2, space="PSUM") as ps,
        ):
            qs = sb.tile([D, BN], MMDT, tag="qs")
            ds = sb.tile([D, DCOLS], MMDT, tag="ds")
            ns = sb.tile([D, DCOLS], MMDT, tag="ns")
            qps = sb.tile([D, CL * N], MMDT, tag="qps")
            maxall = sb.tile([128, 132], F16, tag="maxall")

            # Warm-up chain: memset-backed matmuls, no DMA dependency, keep
            # the PE continuously busy so the p-state ramp finishes before
            # the first real matmul.
            wa = sb.tile([D, 128], F16, tag="wa")
            wb = sb.tile([D, 512], F16, tag="wb")
            nc.gpsimd.memset(wa[:, :], 0.0)
            nc.gpsimd.memset(wb[:, :], 0.0)
            wt = ps.tile([128, 2048], F32, tag="chunk", name="warm")
            for w in range(K_WARM):
                nc.tensor.matmul(
                    wt[:, 512 * (w % 4) : 512 * (w % 4 + 1)],
                    wa[:, :],
                    wb[:, :],
                    start=True,
                    stop=True,
                )

            # Dummy 1-elem scalar copy: forces the implicit ACT_TABLE_LOAD
            # to be inserted here (runs during the DMA phase), so the first
            # real PSUM copy isn't delayed by the 1.28us table load.
            nc.scalar.copy(wb[0:1, 0:1], wa[0:1, 0:1])

            # Input DMAs, priority order.  The HWDGE queues (sync/SP and
            # scalar/ACT) are much faster to issue than gpsimd's SWDGE, so
            # the critical tensors (first query chunks + dT) go there; only
            # the tiny qp rides SWDGE.
            nc.sync.dma_start(out=qs[:, 0:512], in_=qT[:, 0:512])
            nc.scalar.dma_start(out=ds[:, 0:1024], in_=dT[:, 0:1024])
            nc.sync.dma_start(out=ds[:, 1024:2048], in_=dT[:, 1024:2048])
            nc.scalar.dma_start(out=qs[:, 512:1024], in_=qT[:, 512:1024])
            nc.sync.dma_start(out=ns[:, :], in_=nT[:, :])
            nc.gpsimd.dma_start(out=qps[:, :], in_=qp[:, :])
            nc.sync.dma_start(out=qs[:, 1024:2048], in_=qT[:, 1024:2048])

            arenas = {}
            bat_of = {}
            for bi, bb in enumerate(TREE_BATCHES):
                for sl, mm in enumerate(bb):
                    bat_of[mm] = (bi, sl)

            def get_arena(bi):
                if bi not in arenas:
                    arenas[bi] = arp.tile(
                        [128, 8192], F16, tag="arena", name=f"a{bi}"
                    )
                return arenas[bi]

            def emit_mms(m, t):
                bi, slot = bat_of.get(m, (None, None))
                for u in range(4):
                    nc.tensor.matmul(
                        t[:, 512 * u : 512 * (u + 1)],
                        qs[:, 128 * m : 128 * (m + 1)],
                        ds[:, 512 * u : 512 * (u + 1)],
                        start=True,
                        stop=True,
                    )
                    if m == V_HEAD and u == 1:
                        nc.vector.reduce_max(
                            maxall[:, 8 * BLK[m] : 8 * BLK[m] + 4],
                            t[:, 0:1024].rearrange("p (g s) -> p g s", s=S),
                            axis=X,
                        )
                    if m in A_SPLIT and u == 1:
                        nc.scalar.copy(
                            get_arena(bi)[:, 2048 * slot : 2048 * slot + 1024],
                            t[:, 0:1024],
                        )

            def emit_unit(m):
                t = ps.tile([128, 2048], F32, tag="chunk", name=f"u{m}")
                emit_mms(m, t)
                if m == V_HEAD:
                    nc.vector.reduce_max(
                        maxall[:, 8 * BLK[m] + 4 : 8 * BLK[m] + 8],
                        t[:, 1024:2048].rearrange("p (g s) -> p g s", s=S),
                        axis=X,
                    )
                elif m in V_MID:
                    nc.vector.reduce_max(
                        maxall[:, 8 * BLK[m] : 8 * BLK[m] + 8],
                        t[:, :].rearrange("p (g s) -> p g s", s=S),
                        axis=X,
                    )
                elif m in A_SPLIT:
                    bi, slot = bat_of[m]
                    nc.scalar.copy(
                        get_arena(bi)[:, 2048 * slot + 1024 : 2048 * (slot + 1)],
                        t[:, 1024:2048],
                    )
                else:
                    bi, slot = bat_of[m]
                    nc.scalar.copy(
                        get_arena(bi)[:, 2048 * slot : 2048 * (slot + 1)], t[:, :]
                    )

            s1outs = {}

            def get_s1out(bi):
                if bi not in s1outs:
                    s1outs[bi] = trp.tile([128, 4096], F16, tag="t1",
                                          name=f"t1_{bi}")
                return s1outs[bi]

            def emit_tree_s1(bi, sl):
                # stage 1 for one unit of batch bi (per-unit form, used for
                # the trailing batches so each can start as its copy lands)
                gv = arenas[bi][:, 2048 * sl : 2048 * (sl + 1)].rearrange(
                    "p (g s) -> p g s", s=256
                )
                ov = get_s1out(bi)[:, 1024 * sl : 1024 * (sl + 1)].rearrange(
                    "p (g s) -> p g s", s=128
                )
                nc.vector.tensor_max(ov, gv[:, :, 0:128], gv[:, :, 128:256])

            def emit_tree_rest(bi):
                L = len(TREE_BATCHES[bi])
                base = BLK[TREE_BATCHES[bi][0]]
                t1v = get_s1out(bi)[:, 0 : 1024 * L].rearrange(
                    "p (g s) -> p g s", s=128
                )
                t2 = trp.tile([128, 2048], F16, tag="t2")
                t2v = t2[:, 0 : 512 * L].rearrange("p (g s) -> p g s", s=64)
                nc.vector.tensor_max(t2v, t1v[:, :, 0:64], t1v[:, :, 64:128])
                t3 = trp.tile([128, 1024], F16, tag="t3")
                t3v = t3[:, 0 : 256 * L].rearrange("p (g s) -> p g s", s=32)
                nc.vector.tensor_max(t3v, t2v[:, :, 0:32], t2v[:, :, 32:64])
                t4 = trp.tile([128, 512], F16, tag="t4")
                t4v = t4[:, 0 : 128 * L].rearrange("p (g s) -> p g s", s=16)
                nc.vector.tensor_max(t4v, t3v[:, :, 0:16], t3v[:, :, 16:32])
                # drain-order blocks are contiguous: one batched reduce
                nc.vector.reduce_max(
                    maxall[:, 8 * base : 8 * (base + L)], t4v, axis=X
                )

            def emit_tree(bi):
                # batched stage 1: one op over the whole arena
                L = len(TREE_BATCHES[bi])
                gv = arenas[bi][:, 0 : 2048 * L].rearrange(
                    "p (g s) -> p g s", s=256
                )
                ov = get_s1out(bi)[:, 0 : 1024 * L].rearrange(
                    "p (g s) -> p g s", s=128
                )
                nc.vector.tensor_max(ov, gv[:, :, 0:128], gv[:, :, 128:256])
                emit_tree_rest(bi)

            def emit_pairwise():
                pt = ps.tile([128, 2048], F32, tag="chunk", name="ptw")
                for b in range(CL):
                    g, j = divmod(b, 4)
                    nc.tensor.matmul(
                        pt[32 * j : 32 * (j + 1), 256 * g : 256 * (g + 1)],
                        qps[:, 32 * b : 32 * (b + 1)],
                        ns[:, 256 * b : 256 * (b + 1)],
                        start=True,
                        stop=True,
                        tile_position=(0, 32 * j),
                    )
                nc.vector.reduce_max(
                    maxall[:, 128:130],
                    pt[:, 0:512].rearrange("p (g s) -> p g s", s=S),
                    axis=X,
                )

            # ---- emission order (PE + engine queues are FIFO) ----------
            emit_unit(0)          # A-split: ACT starts after 2 of 4 MMs
            emit_unit(1)          # A-split
            emit_unit(2)          # A
            emit_pairwise()       # early DVE filler (needs ns+qps)
            emit_unit(3)          # Vh: halved DVE reduce
            emit_tree(0)          # B0 {0,1,2}
            for m in (4, 5, 6):
                emit_unit(m)
            emit_tree(1)          # B1 {4,5,6}
            for m in (7, 8):      # A, V-mid
                emit_unit(m)
            for m in (9, 10):
                emit_unit(m)
            emit_tree(2)          # B2 {7,9,10}
            nc.sync.dma_start(out=out_d[:, 0:64], in_=maxall[:, 0:64])
            emit_unit(11)
            emit_tree_s1(3, 0)
            emit_unit(12)
            emit_tree_s1(3, 1)
            emit_unit(13)         # V-mid fills DVE while c14 runs
            emit_unit(14)
            emit_tree_s1(3, 2)
            emit_tree_rest(3)     # B3 {11,12,14} closes before last copy
            emit_unit(15)
            emit_tree(4)          # solo {15}: ~1.5us tail after last copy
            nc.sync.dma_start(out=out_d[:, 64:132], in_=maxall[:, 64:132])

    nc.finalize()
    return nc


LAST_RESULT = None


def kernel(query_embeddings, doc_embeddings, neg_doc_embeddings):
    global LAST_RESULT
    _install_ntff_shim()

    q = np.asarray(query_embeddings, dtype=np.float32)
    d = np.asarray(doc_embeddings, dtype=np.float32)
    g = np.asarray(neg_doc_embeddings, dtype=np.float32)
    assert q.shape == (B, N, D) and d.shape == (B, S, D) and g.shape == (B, S, D)

    qT_all = np.ascontiguousarray(
        q.transpose(2, 0, 1).reshape(D, BN).astype(np.float16)
    )

    in_maps = []
    for k in range(NC):
        dT_k = np.ascontiguousarray(
            d[CL * k : CL * (k + 1)].transpose(2, 0, 1).reshape(D, DCOLS)
            .astype(np.float16)
        )
        nT_k = np.ascontiguousarray(
            g[CL * k : CL * (k + 1)].transpose(2, 0, 1).reshape(D, DCOLS)
            .astype(np.float16)
        )
        qp_k = np.ascontiguousarray(qT_all[:, CL * N * k : CL * N * (k + 1)])
        in_maps.append({"qT": qT_all, "dT": dT_k, "nT": nT_k, "qp": qp_k})

    if "nc" not in _CACHE:
        _CACHE["nc"] = _build()
    res = run_bass_kernel_spmd(_CACHE["nc"], in_maps, core_ids=list(range(NC)))
    LAST_RESULT = res

    # Host epilogue: n-sum, assembly, softplus means.
    scores = np.empty((B, B), dtype=np.float64)
    negpair = np.empty((B,), dtype=np.float64)
    for k in range(NC):
        o = res.results[k]["out"].astype(np.float64)  # (128, 132)
        for m in range(16):
            # rows 32j+n, block col 8*BLK[m]+c  ->  scores[4m+j, CL*k+c]
            b8 = 8 * BLK[m]
            blk = o[:, b8 : b8 + 8].reshape(4, N, CL).sum(axis=1)
            scores[4 * m : 4 * m + 4, CL * k : CL * (k + 1)] = blk
        pw = o[:, 128:130].reshape(4, N, 2).sum(axis=1)  # rows j, col g
        for gcol in range(2):
            for j in range(4):
                negpair[CL * k + 4 * gcol + j] = pw[j, gcol]

    pos = np.diagonal(scores)
    l1 = np.logaddexp(0.0, negpair - pos).mean()
    neg_ib = (scores - np.eye(B, dtype=np.float64) * NEG_INF_DIAG).max(axis=1)
    l2 = np.logaddexp(0.0, neg_ib - pos).mean()
    return np.asarray((l1 + l2) / 2.0, dtype=np.float32)
